# revision 40
# baseline (speedup 1.0000x reference)
"""AdaPool3d Trainium2 kernel — 8-core data parallel.

x [4,64,16,112,112] f32, beta [8,56,56] f32 -> out [4,64,8,56,56] f32.
256 (b,c) images sharded 32/core. Per image [16,112,112]:
  SBUF layout X [128, 1568] bf16 (DMA-cast): partition p=(kd,od,oh3),
  free f = ohp*224 + kh*112 + (2*ow+kw);  d=2*od+kd, h=16*ohp+2*oh3+kh.
Window sums via PE pooling matmuls (4 strided rhs accumulated), Dice
reciprocal via one fused custom-DVE op, softmax exps on ACT.
"""

import os
import numpy as np

_NCORES = 8
_IMGS = 32          # images per core
_D, _H, _W = 16, 112, 112
_OD, _OH, _OW = 8, 56, 56
_OHP, _OH3 = 7, 8   # oh = 8*ohp + oh3
_FD = 1568          # per-partition free elems = 7*224
_NW = 392           # windows per q-group = 7*56

_cache = {}


def _register_op(name, spec):
    from concourse.dve_spec import lower, _has_src1
    from concourse import dve_ops
    from concourse.dve_uop import DveOpSpec

    for op in dve_ops.OPS:
        if op.name == name:
            return op
    row = dve_ops._CUSTOM_DVE_ROW_BASE + len(dve_ops.OPS)
    assert row < 0x20
    dve_ops._SUB_OPCODE_FOR_NAME[name] = row
    shas = {}
    for ver in ("v3", "v4"):
        try:
            uops = lower(spec, ver=ver)
            shas[ver] = DveOpSpec(
                name=name, opcode=row, uops=uops, rd1_en=_has_src1(spec)
            ).sha(ver)
        except Exception:
            pass
    op = dve_ops.DveOp(name, spec, subdim=False, uops_sha=shas)
    dve_ops.OPS.append(op)
    dve_ops.CUSTOM_DVE_SPECS[name] = spec
    return op


def _register_custom_ops():
    """DICE: u*approx(1/(1+u^2)); RECIP1_EPS: approx(1/(x+eps)) 1-NR;
    DIV1: num*approx(1/den) 1-NR."""
    from concourse.dve_spec import Spec, Src0, Src1, Bin, AluOp, sq, One, C0, C1, C2

    def _r1(z, s0, s1):
        nb = (~z.view(np.int32)).view(np.float32)
        y0 = nb * np.float32(s0)
        return y0 * (np.float32(s1) - z * y0)

    _z = sq(Src0) + One
    _nb = Bin(AluOp.BITWISE_NOT, _z, _z)
    _y0 = _nb * C0
    _y1 = _y0 * (C1 - _z * _y0)
    dice = _register_op(
        "DICE_ANT",
        Spec(
            body=Src0 * _y1,
            reference=lambda in0, in1, s0, s1, imm2: in0
            * _r1((1.0 + in0.astype(np.float32) ** 2), s0, s1),
        ),
    )

    _ze = Src0 + C2
    _nbe = Bin(AluOp.BITWISE_NOT, _ze, _ze)
    _y0e = _nbe * C0
    _y1e = _y0e * (C1 - _ze * _y0e)
    recip1 = _register_op(
        "RECIP1_EPS_ANT",
        Spec(
            body=_y1e,
            reference=lambda in0, in1, s0, s1, imm2: _r1(
                in0.astype(np.float32) + np.float32(imm2), s0, s1
            ),
        ),
    )

    _nbd = Bin(AluOp.BITWISE_NOT, Src1, Src1)
    _y0d = _nbd * C0
    _y1d = _y0d * (C1 - Src1 * _y0d)
    div1 = _register_op(
        "DIV1_ANT",
        Spec(
            body=Src0 * _y1d,
            reference=lambda in0, in1, s0, s1, imm2: in0
            * _r1(in1.astype(np.float32), s0, s1),
        ),
    )
    return dice, recip1, div1


_C0, _C1 = -0.23549792, 2.0017324


def _build():
    if "nc" in _cache:
        return _cache["nc"]
    import concourse.bass as bass
    import concourse.bacc as bacc
    import concourse.mybir as mybir
    from concourse.tile import TileContext
    from contextlib import ExitStack

    DICE, RECIP1, DIV1 = _register_custom_ops()
    f32, bf16 = mybir.dt.float32, mybir.dt.bfloat16
    AF = mybir.ActivationFunctionType
    MUL, ADD, SUB = (
        mybir.AluOpType.mult,
        mybir.AluOpType.add,
        mybir.AluOpType.subtract,
    )

    nc = bacc.Bacc(None, target_bir_lowering=False, debug=False)
    # host pre-rearranged: x [img, p=(kd,od,oh3), f=(ohp,kh,w)]
    x_d = nc.dram_tensor("x", [_IMGS, 128, _FD], f32, kind="ExternalInput")
    beta_d = nc.dram_tensor("beta", [64, _NW], f32, kind="ExternalInput")
    lhs_d = nc.dram_tensor("lhs", [128, 192], f32, kind="ExternalInput")
    out_d = nc.dram_tensor("out", [_IMGS, 64, _NW], f32, kind="ExternalOutput")

    x_ap = x_d.ap()
    out_ap = out_d.ap()
    beta_v = beta_d.ap()

    with TileContext(nc) as tc, ExitStack() as ctx:
        const = ctx.enter_context(tc.tile_pool(name="const", bufs=1))
        xin = ctx.enter_context(tc.tile_pool(name="xin", bufs=4))
        big = ctx.enter_context(tc.tile_pool(name="big", bufs=3))
        sm = ctx.enter_context(tc.tile_pool(name="sm", bufs=3))
        ps = ctx.enter_context(tc.tile_pool(name="ps", bufs=1, space="PSUM"))
        pss = ctx.enter_context(tc.tile_pool(name="pss", bufs=1, space="PSUM"))

        # Constants: one shared pooling lhsT (cols 128:192, 1.0 at p%64==q)
        lhs_t = const.tile([128, 192], bf16, name="lhsT")
        nc.gpsimd.dma_start(out=lhs_t[:], in_=lhs_d.ap())
        lhsT_sum = lhs_t[:, 128:192]
        beta_t = const.tile([64, _NW], f32, name="betat")
        nc.sync.dma_start(out=beta_t[:], in_=beta_v)

        for i in range(_IMGS):
            # ---- load image (cast f32->bf16); free = (ohp, ow, kh, kw)
            X = xin.tile([128, _FD], bf16, tag="X")
            nc.gpsimd.dma_start(out=X[:], in_=x_ap[i])
            Xf = X[:]
            X4 = X[:].rearrange("p (ab c) -> p ab c", c=4)  # [128, 392, 4]

            # ---- sum pool: 4 strided matmuls -> PSUM [64, 392] (shared lhsT)
            pA = ps.tile([64, _NW], f32, tag="pA")
            for j in range(4):
                nc.tensor.matmul(
                    pA[:], lhsT_sum, X4[:, :, j],
                    start=(j == 0), stop=(j == 3),
                )

            # ---- recipA = 8/(Sx+eps); kd-dup to upper partitions via DMA
            rA = sm.tile([128, _NW], f32, tag="rA")
            nc.vector._custom_dve(
                RECIP1, out=rA[0:64, :], in0=pA[:], s0=_C0, s1=_C1, imm2=1e-12
            )
            nc.sync.dma_start(out=rA[64:128, :], in_=rA[0:64, :])

            # ---- u = X * broadcast(rA) ; dsc = DICE(u)
            rA_b = (
                rA[:]
                .rearrange("p (ab u) -> p ab u", u=1)
                .broadcast_to([128, _NW, 4])
            )
            # u = (8*x) * recip(Sx)  (the 8 folds the mean's /8; exact in bf16)
            U = big.tile([128, _FD], bf16, tag="U")
            Uv = U[:].rearrange("p (ab c) -> p ab c", c=4)
            nc.vector.scalar_tensor_tensor(
                Uv, X4, 8.0, rA_b, op0=MUL, op1=MUL
            )
            DS = big.tile([128, _FD], bf16, tag="DS")
            nc.vector._custom_dve(DICE, out=DS[:], in0=U[:], s0=_C0, s1=_C1)
            E = big.tile([128, _FD], bf16, tag="E")
            nc.scalar.activation(E[:], Xf, AF.Exp)
            F = big.tile([128, _FD], bf16, tag="F")
            nc.scalar.activation(F[:], DS[:], AF.Exp, scale=2.0)

            # ---- products
            M_ = big.tile([128, _FD], bf16, tag="M")
            nc.vector.tensor_tensor(M_[:], E[:], Xf, op=MUL)
            FX = big.tile([128, _FD], bf16, tag="FX")
            nc.vector.tensor_tensor(FX[:], F[:], Xf, op=MUL)

            # ---- window sums of e, m, f, fx -> PSUM [64, 392] each
            psums = {}
            for nm, T in (("e", E), ("m", M_), ("f", F), ("x", FX)):
                pT = pss.tile([64, _NW], f32, tag="p" + nm)
                Tv = T[:].rearrange("p (ab c) -> p ab c", c=4)
                for j in range(4):
                    nc.tensor.matmul(
                        pT[:], lhsT_sum, Tv[:, :, j],
                        start=(j == 0), stop=(j == 3),
                    )
                psums[nm] = pT

            # ---- combine: out = em + beta*(edscw - em)
            # (one PSUM read per instruction: denominators go to SBUF first)
            se_s = sm.tile([64, _NW], f32, tag="se_s")
            nc.vector.tensor_copy(se_s[:], psums["e"][:])
            sf_s = sm.tile([64, _NW], f32, tag="sf_s")
            nc.vector.tensor_copy(sf_s[:], psums["f"][:])
            em = sm.tile([64, _NW], f32, tag="em")
            nc.vector._custom_dve(
                DIV1, out=em[:], in0=psums["m"][:], in1=se_s[:],
                s0=_C0, s1=_C1,
            )
            ed = sm.tile([64, _NW], f32, tag="ed")
            nc.vector._custom_dve(
                DIV1, out=ed[:], in0=psums["x"][:], in1=sf_s[:],
                s0=_C0, s1=_C1,
            )
            dd = sm.tile([64, _NW], f32, tag="dd")
            nc.gpsimd.tensor_tensor(dd[:], ed[:], em[:], op=SUB)
            tt = sm.tile([64, _NW], f32, tag="tt")
            nc.gpsimd.tensor_tensor(tt[:], beta_t[:], dd[:], op=MUL)
            oc = sm.tile([64, _NW], f32, tag="oc")
            nc.gpsimd.tensor_tensor(oc[:], em[:], tt[:], op=ADD)

            # ---- store
            nc.sync.dma_start(out=out_ap[i], in_=oc[:])

    nc.finalize()
    _cache["nc"] = nc
    return nc


def _lhs_const():
    lhs = np.zeros((128, 192), np.float32)
    for p in range(128):
        q = p % 64
        lhs[p, q] = 0.125          # avg cols m in [0,128): m%64==q (dup)
        lhs[p, 64 + q] = 0.125
        lhs[p, 128 + q] = 1.0      # sum cols
    return lhs


def _prep_x(x, n):
    # [img, kd, od, oh3, ohp, ow, kh, kw] -> [img, 128, 1568]
    return np.ascontiguousarray(
        x.reshape(n, 8, 2, _OHP, _OH3, 2, 56, 2)
        .transpose(0, 2, 1, 4, 3, 6, 5, 7)
        .reshape(n, 128, _FD)
    )


def _prep_beta(beta):
    # beta [8,56,56] -> [q=(od,oh3), (ohp,ow)]
    return np.ascontiguousarray(
        beta.reshape(_OD, _OHP, _OH3, _OW).transpose(0, 2, 1, 3).reshape(64, _NW)
    )


def _unprep_out(outs, B, C):
    n = B * C
    return np.ascontiguousarray(
        outs.reshape(n, _OD, _OH3, _OHP, _OW)
        .transpose(0, 1, 3, 2, 4)
        .reshape(B, C, _OD, _OH, _OW)
    )


def kernel(**inputs):
    x = np.asarray(inputs["x"], dtype=np.float32)
    beta = np.asarray(inputs["beta"], dtype=np.float32)
    B, C = x.shape[0], x.shape[1]
    n = B * C
    x_r = _prep_x(x, n)
    beta_r = _prep_beta(beta)
    nc = _build()
    lhs = _lhs_const()
    in_maps = [
        {
            "x": np.ascontiguousarray(x_r[i * _IMGS : (i + 1) * _IMGS]),
            "beta": beta_r,
            "lhs": lhs,
        }
        for i in range(_NCORES)
    ]
    from concourse.bass_utils import run_bass_kernel_spmd

    res = run_bass_kernel_spmd(nc, in_maps, core_ids=list(range(_NCORES)))
    outs = np.stack([np.asarray(res.results[i]["out"]) for i in range(_NCORES)])
    return _unprep_out(outs, B, C)


if __name__ == "__main__":
    _build()
    print("build OK")


# revision 43
# speedup vs baseline: 1.1600x; 1.1600x over previous
"""AdaPool3d Trainium2 kernel — 8-core data parallel.

x [4,64,16,112,112] f32, beta [8,56,56] f32 -> out [4,64,8,56,56] f32.
256 (b,c) images sharded 32/core. Per image [16,112,112]:
  SBUF layout X [128, 1568] bf16 (DMA-cast): partition p=(kd,od,oh3),
  free f = ohp*224 + kh*112 + (2*ow+kw);  d=2*od+kd, h=16*ohp+2*oh3+kh.
Window sums via PE pooling matmuls (4 strided rhs accumulated), Dice
reciprocal via one fused custom-DVE op, softmax exps on ACT.
"""

import os
import numpy as np

_NCORES = 8
_IMGS = 32          # images per core
_D, _H, _W = 16, 112, 112
_OD, _OH, _OW = 8, 56, 56
_OHP, _OH3 = 7, 8   # oh = 8*ohp + oh3
_FD = 1568          # per-partition free elems = 7*224
_NW = 392           # windows per q-group = 7*56

_cache = {}


def _register_op(name, spec):
    from concourse.dve_spec import lower, _has_src1
    from concourse import dve_ops
    from concourse.dve_uop import DveOpSpec

    for op in dve_ops.OPS:
        if op.name == name:
            return op
    row = dve_ops._CUSTOM_DVE_ROW_BASE + len(dve_ops.OPS)
    assert row < 0x20
    dve_ops._SUB_OPCODE_FOR_NAME[name] = row
    shas = {}
    for ver in ("v3", "v4"):
        try:
            uops = lower(spec, ver=ver)
            shas[ver] = DveOpSpec(
                name=name, opcode=row, uops=uops, rd1_en=_has_src1(spec)
            ).sha(ver)
        except Exception:
            pass
    op = dve_ops.DveOp(name, spec, subdim=False, uops_sha=shas)
    dve_ops.OPS.append(op)
    dve_ops.CUSTOM_DVE_SPECS[name] = spec
    return op


def _register_custom_ops():
    """DICE: u*approx(1/(1+u^2)); RECIP1_EPS: approx(1/(x+eps)) 1-NR;
    DIV1: num*approx(1/den) 1-NR."""
    from concourse.dve_spec import Spec, Src0, Src1, Bin, AluOp, sq, One, C0, C1, C2

    def _r1(z, s0, s1):
        nb = (~z.view(np.int32)).view(np.float32)
        y0 = nb * np.float32(s0)
        return y0 * (np.float32(s1) - z * y0)

    _z = sq(Src0) + One
    _nb = Bin(AluOp.BITWISE_NOT, _z, _z)
    _y0 = _nb * C0
    _y1 = _y0 * (C1 - _z * _y0)
    dice = _register_op(
        "DICE_ANT",
        Spec(
            body=Src0 * _y1,
            reference=lambda in0, in1, s0, s1, imm2: in0
            * _r1((1.0 + in0.astype(np.float32) ** 2), s0, s1),
        ),
    )

    _ze = Src0 + C2
    _nbe = Bin(AluOp.BITWISE_NOT, _ze, _ze)
    _y0e = _nbe * C0
    _y1e = _y0e * (C1 - _ze * _y0e)
    recip1 = _register_op(
        "RECIP1_EPS_ANT",
        Spec(
            body=_y1e,
            reference=lambda in0, in1, s0, s1, imm2: _r1(
                in0.astype(np.float32) + np.float32(imm2), s0, s1
            ),
        ),
    )

    _nbd = Bin(AluOp.BITWISE_NOT, Src1, Src1)
    _y0d = _nbd * C0
    _y1d = _y0d * (C1 - Src1 * _y0d)
    div1 = _register_op(
        "DIV1_ANT",
        Spec(
            body=Src0 * _y1d,
            reference=lambda in0, in1, s0, s1, imm2: in0
            * _r1(in1.astype(np.float32), s0, s1),
        ),
    )
    return dice, recip1, div1


_C0, _C1 = -0.23549792, 2.0017324


def _build():
    if "nc" in _cache:
        return _cache["nc"]
    import concourse.bass as bass
    import concourse.bacc as bacc
    import concourse.mybir as mybir
    from concourse.tile import TileContext
    from contextlib import ExitStack

    DICE, RECIP1, DIV1 = _register_custom_ops()
    f32, bf16 = mybir.dt.float32, mybir.dt.bfloat16
    AF = mybir.ActivationFunctionType
    MUL, ADD, SUB = (
        mybir.AluOpType.mult,
        mybir.AluOpType.add,
        mybir.AluOpType.subtract,
    )

    nc = bacc.Bacc(None, target_bir_lowering=False, debug=False)
    # host pre-rearranged: x [img, p=(kd,od,oh3), f=(ohp,kh,w)]
    x_d = nc.dram_tensor("x", [_IMGS, 128, _FD], f32, kind="ExternalInput")
    beta_d = nc.dram_tensor("beta", [64, _NW], f32, kind="ExternalInput")
    lhs_d = nc.dram_tensor("lhs", [128, 192], f32, kind="ExternalInput")
    out_d = nc.dram_tensor("out", [_IMGS, 64, _NW], f32, kind="ExternalOutput")

    x_ap = x_d.ap()
    out_ap = out_d.ap()
    beta_v = beta_d.ap()

    with TileContext(nc) as tc, ExitStack() as ctx:
        const = ctx.enter_context(tc.tile_pool(name="const", bufs=1))
        xin = ctx.enter_context(tc.tile_pool(name="xin", bufs=4))
        big = ctx.enter_context(tc.tile_pool(name="big", bufs=3))
        sm = ctx.enter_context(tc.tile_pool(name="sm", bufs=3))
        ps = ctx.enter_context(tc.tile_pool(name="ps", bufs=1, space="PSUM"))
        pss = ctx.enter_context(tc.tile_pool(name="pss", bufs=1, space="PSUM"))

        # Constants: lhsT (avg cols 0:128 @0.125-dup, sum cols 128:192 @1.0)
        lhs_t = const.tile([128, 192], bf16, name="lhsT")
        nc.gpsimd.dma_start(out=lhs_t[:], in_=lhs_d.ap())
        lhsT_avg = lhs_t[:, 0:128]
        lhsT_sum = lhs_t[:, 128:192]
        beta_t = const.tile([64, _NW], f32, name="betat")
        nc.sync.dma_start(out=beta_t[:], in_=beta_v)

        for i in range(_IMGS):
            # ---- load image (cast f32->bf16); free = (ohp, ow, kh, kw)
            X = xin.tile([128, _FD], bf16, tag="X")
            nc.gpsimd.dma_start(out=X[:], in_=x_ap[i])
            Xf = X[:]
            X4 = X[:].rearrange("p (ab c) -> p ab c", c=4)  # [128, 392, 4]

            # ---- avg pool: 4 strided matmuls -> PSUM [128, 392] (kd-dup)
            pA = ps.tile([128, _NW], f32, tag="pA")
            for j in range(4):
                nc.tensor.matmul(
                    pA[:], lhsT_avg, X4[:, :, j],
                    start=(j == 0), stop=(j == 3),
                )

            # ---- recipA (eps inside op so zero window sums don't NaN)
            rA = sm.tile([128, _NW], f32, tag="rA")
            nc.vector._custom_dve(
                RECIP1, out=rA[:], in0=pA[:], s0=_C0, s1=_C1, imm2=1e-12
            )

            # ---- u = X * broadcast(rA) ; dsc = DICE(u)
            rA_b = (
                rA[:]
                .rearrange("p (ab u) -> p ab u", u=1)
                .broadcast_to([128, _NW, 4])
            )
            U = big.tile([128, _FD], bf16, tag="U")
            Uv = U[:].rearrange("p (ab c) -> p ab c", c=4)
            nc.vector.tensor_tensor(Uv, X4, rA_b, op=MUL)
            DS = big.tile([128, _FD], bf16, tag="DS")
            nc.vector._custom_dve(DICE, out=DS[:], in0=U[:], s0=_C0, s1=_C1)
            E = big.tile([128, _FD], bf16, tag="E")
            nc.scalar.activation(E[:], Xf, AF.Exp)
            F = big.tile([128, _FD], bf16, tag="F")
            nc.scalar.activation(F[:], DS[:], AF.Exp, scale=2.0)

            # ---- products
            M_ = big.tile([128, _FD], bf16, tag="M")
            nc.vector.tensor_tensor(M_[:], E[:], Xf, op=MUL)
            FX = big.tile([128, _FD], bf16, tag="FX")
            nc.vector.tensor_tensor(FX[:], F[:], Xf, op=MUL)

            # ---- window sums of e, m, f, fx -> PSUM [64, 392] each
            psums = {}
            for nm, T in (("e", E), ("m", M_), ("f", F), ("x", FX)):
                pT = pss.tile([64, _NW], f32, tag="p" + nm)
                Tv = T[:].rearrange("p (ab c) -> p ab c", c=4)
                for j in range(4):
                    nc.tensor.matmul(
                        pT[:], lhsT_sum, Tv[:, :, j],
                        start=(j == 0), stop=(j == 3),
                    )
                psums[nm] = pT

            # ---- combine: out = em + beta*(edscw - em)
            # (one PSUM read per instruction: denominators go to SBUF first)
            se_s = sm.tile([64, _NW], f32, tag="se_s")
            nc.vector.tensor_copy(se_s[:], psums["e"][:])
            sf_s = sm.tile([64, _NW], f32, tag="sf_s")
            nc.vector.tensor_copy(sf_s[:], psums["f"][:])
            em = sm.tile([64, _NW], f32, tag="em")
            nc.vector._custom_dve(
                DIV1, out=em[:], in0=psums["m"][:], in1=se_s[:],
                s0=_C0, s1=_C1,
            )
            ed = sm.tile([64, _NW], f32, tag="ed")
            nc.vector._custom_dve(
                DIV1, out=ed[:], in0=psums["x"][:], in1=sf_s[:],
                s0=_C0, s1=_C1,
            )
            dd = sm.tile([64, _NW], f32, tag="dd")
            nc.gpsimd.tensor_tensor(dd[:], ed[:], em[:], op=SUB)
            tt = sm.tile([64, _NW], f32, tag="tt")
            nc.gpsimd.tensor_tensor(tt[:], beta_t[:], dd[:], op=MUL)
            oc = sm.tile([64, _NW], f32, tag="oc")
            nc.gpsimd.tensor_tensor(oc[:], em[:], tt[:], op=ADD)

            # ---- store
            nc.sync.dma_start(out=out_ap[i], in_=oc[:])

    nc.finalize()
    _cache["nc"] = nc
    return nc


def _lhs_const():
    lhs = np.zeros((128, 192), np.float32)
    for p in range(128):
        q = p % 64
        lhs[p, q] = 0.125          # avg cols m in [0,128): m%64==q (dup)
        lhs[p, 64 + q] = 0.125
        lhs[p, 128 + q] = 1.0      # sum cols
    return lhs


def _prep_x(x, n):
    # [img, kd, od, oh3, ohp, ow, kh, kw] -> [img, 128, 1568]
    return np.ascontiguousarray(
        x.reshape(n, 8, 2, _OHP, _OH3, 2, 56, 2)
        .transpose(0, 2, 1, 4, 3, 6, 5, 7)
        .reshape(n, 128, _FD)
    )


def _prep_beta(beta):
    # beta [8,56,56] -> [q=(od,oh3), (ohp,ow)]
    return np.ascontiguousarray(
        beta.reshape(_OD, _OHP, _OH3, _OW).transpose(0, 2, 1, 3).reshape(64, _NW)
    )


def _unprep_out(outs, B, C):
    n = B * C
    return np.ascontiguousarray(
        outs.reshape(n, _OD, _OH3, _OHP, _OW)
        .transpose(0, 1, 3, 2, 4)
        .reshape(B, C, _OD, _OH, _OW)
    )


def kernel(**inputs):
    x = np.asarray(inputs["x"], dtype=np.float32)
    beta = np.asarray(inputs["beta"], dtype=np.float32)
    B, C = x.shape[0], x.shape[1]
    n = B * C
    x_r = _prep_x(x, n)
    beta_r = _prep_beta(beta)
    nc = _build()
    lhs = _lhs_const()
    in_maps = [
        {
            "x": np.ascontiguousarray(x_r[i * _IMGS : (i + 1) * _IMGS]),
            "beta": beta_r,
            "lhs": lhs,
        }
        for i in range(_NCORES)
    ]
    from concourse.bass_utils import run_bass_kernel_spmd

    res = run_bass_kernel_spmd(nc, in_maps, core_ids=list(range(_NCORES)))
    outs = np.stack([np.asarray(res.results[i]["out"]) for i in range(_NCORES)])
    return _unprep_out(outs, B, C)


if __name__ == "__main__":
    _build()
    print("build OK")
